# revision 9
# baseline (speedup 1.0000x reference)
"""Bayesian linear layer on 8 TRN2 NeuronCores.

Math: W = weight_mu + softplus(weight_rho) * weight_epsilon   [O, I]
      b = bias_mu  + softplus(bias_rho)  * bias_epsilon       [O]
      out = x @ W.T + b                                       [T, O]

Sharding: column-parallel — each core owns O/8 = 512 out_features.
x is replicated; no collectives. Host pre-transposes x and the weight
params to I-major layout so every DMA is a natural contiguous load and
the contraction dim lands on SBUF partitions with zero on-chip
transposes.

Per-core kernel: cache W^T (constructed on-chip from mu/rho/eps) in
SBUF, stream x^T tiles, accumulate psum[T=128, O=512] over K=4096.
"""

import numpy as np

import concourse.bass as bass
import concourse.mybir as mybir
import concourse.tile as tile
from concourse import bacc
from concourse.bass import ds, ts


def _ensure_axon_hooks():
    """concourse's trace path imports antenv.axon_hooks, which this image
    lacks. Synthesize it and register the ctypes NTFF hook so profiling
    works (and trace=True doesn't crash)."""
    try:
        import antenv.axon_hooks  # noqa: F401

        return
    except ImportError:
        pass
    import sys
    import types

    mod = types.ModuleType("antenv.axon_hooks")
    mod._hook = None
    mod.set_axon_ntff_profile_hook = lambda h: setattr(mod, "_hook", h)
    mod.get_axon_ntff_profile_hook = lambda: mod._hook
    try:
        import antenv

        antenv.axon_hooks = mod
    except ImportError:
        pass
    sys.modules["antenv.axon_hooks"] = mod
    try:
        import os

        if os.path.exists("/opt/axon/libaxon_pjrt.so"):
            sys.path.insert(0, "/root/.axon_site")
            from trn_agent_boot.trn_boot import _ntff_profile_via_ctypes

            hook = _ntff_profile_via_ctypes("/opt/axon/libaxon_pjrt.so")
            if hook is not None:
                mod.set_axon_ntff_profile_hook(hook)
    except Exception:
        pass


_ensure_axon_hooks()

from concourse.bass_utils import run_bass_kernel_spmd  # noqa: E402

P = 128
TOKENS = 4096
IN_F = 4096
OUT_F = 4096
NCORES = 8

# matmul dtype: "bf16" | "f32r" | "f32"
MM_MODE = "f32r"


def build_nc(
    mm_mode: str = MM_MODE,
    tokens: int = TOKENS,
    in_f: int = IN_F,
    o_shard: int = OUT_F // NCORES,
    kc_chunks: int = 4,
    tchunk: int = 512,
):
    f32 = mybir.dt.float32
    if mm_mode == "bf16":
        # x and weight params pre-cast to bf16 on host; W built in bf16.
        x_dt = mybir.dt.bfloat16
        wp_dt = mybir.dt.bfloat16
        wt_dt = mybir.dt.bfloat16
    elif mm_mode == "f32r":
        # fp32r = fp32 RNE-rounded to 11-bit mantissa, low 12 bits zero.
        # x pre-rounded on host and DMA'd raw; W written as float32r by DVE
        # (engine rounds on write). Full bf16-rate matmul for N>=256.
        x_dt = mybir.dt.float32r
        wp_dt = f32
        wt_dt = mybir.dt.float32r
    elif mm_mode == "f32":
        x_dt = f32
        wp_dt = f32
        wt_dt = f32
    else:
        raise ValueError(mm_mode)

    ko = in_f // P          # total k-subtiles
    assert ko % kc_chunks == 0
    ko_per_kc = ko // kc_chunks
    assert tchunk % P == 0
    tsub_n = tchunk // P
    assert tokens % tchunk == 0
    m4_n = tokens // tchunk
    AF = mybir.ActivationFunctionType

    nc = bacc.Bacc(None, target_bir_lowering=False, debug=False)
    xT = nc.declare_dram_parameter("xT", [in_f, tokens], x_dt, False)
    wp = nc.declare_dram_parameter("wp", [in_f, 3, o_shard], wp_dt, False)
    bp = nc.declare_dram_parameter("bp", [P, 3, o_shard], f32, False)
    out = nc.declare_dram_parameter("out", [tokens, o_shard], f32, True)

    with tile.TileContext(nc) as tc:
        with (
            tc.tile_pool(name="wt", bufs=1) as wt_pool,
            tc.tile_pool(name="wload", bufs=3) as wload_pool,
            tc.tile_pool(name="wtmp", bufs=3) as wtmp_pool,
            tc.tile_pool(name="xload", bufs=3) as x_pool,
            tc.tile_pool(name="biasp", bufs=1) as bias_pool,
            tc.tile_pool(name="outp", bufs=4) as out_pool,
            tc.tile_pool(name="psum", bufs=2, space="PSUM") as psum_pool,
        ):
            # ---- bias = bmu + softplus(brho) * beps, pre-broadcast on 128 partitions
            bload = bias_pool.tile([P, 3, o_shard], f32, name="bload")
            nc.sync.dma_start(out=bload[:], in_=bp[:])
            # softplus(v) = ln(1 + exp(v)): Exp then Ln(in + 1) — both live
            # in the natural_log_exp_and_others ACT table.
            bexp = bias_pool.tile([P, o_shard], f32, name="bexp")
            nc.scalar.activation(bexp[:], bload[:, 1, :], AF.Exp)
            bsig = bias_pool.tile([P, o_shard], f32, name="bsig")
            nc.scalar.activation(bsig[:], bexp[:], AF.Ln, bias=1.0)
            btmp = bias_pool.tile([P, o_shard], f32, name="btmp")
            nc.vector.tensor_mul(btmp[:], bsig[:], bload[:, 2, :])
            bias_bc = bias_pool.tile([P, o_shard], f32, name="bias_bc")
            nc.vector.tensor_add(bias_bc[:], btmp[:], bload[:, 0, :])

            # ---- W^T construction: cached in SBUF for the whole kernel
            wp_r = wp.rearrange("(ko p) c o -> ko p c o", p=P)  # [ko, P, 3, O]
            wt_tiles = []
            for k in range(ko):
                wl = wload_pool.tile([P, 3, o_shard], wp_dt, name="wl")
                nc.sync.dma_start(out=wl[:], in_=wp_r[k])
                sige = wtmp_pool.tile([P, o_shard], f32, name="sige")
                nc.scalar.activation(sige[:], wl[:, 1, :], AF.Exp)
                sig = wtmp_pool.tile([P, o_shard], f32, name="sig")
                nc.scalar.activation(sig[:], sige[:], AF.Ln, bias=1.0)
                tmp = wtmp_pool.tile([P, o_shard], f32, name="tmp")
                nc.vector.tensor_mul(tmp[:], sig[:], wl[:, 2, :])
                wt = wt_pool.tile([P, o_shard], wt_dt, name=f"wt{k}")
                nc.vector.tensor_add(wt[:], tmp[:], wl[:, 0, :])
                wt_tiles.append(wt)

            # ---- main loop: psum[t 128, o 512] accumulated over all of K
            xT_r = xT.rearrange("(a p) t -> p a t", p=P)  # [P, ko, tokens]
            for m4 in range(m4_n):
                psums = [
                    psum_pool.tile([P, o_shard], f32, name=f"ps{i}")
                    for i in range(tsub_n)
                ]
                for kc in range(kc_chunks):
                    xt = x_pool.tile([P, ko_per_kc, tchunk], x_dt, name="xt")
                    nc.sync.dma_start(
                        out=xt[:],
                        in_=xT_r[
                            :,
                            kc * ko_per_kc : (kc + 1) * ko_per_kc,
                            m4 * tchunk : (m4 + 1) * tchunk,
                        ],
                    )
                    for t_sub in range(tsub_n):
                        for k in range(ko_per_kc):
                            lhs = xt[:, k, ts(t_sub, P)]
                            rhs = wt_tiles[kc * ko_per_kc + k][:]
                            nc.tensor.matmul(
                                psums[t_sub][:],
                                lhsT=lhs,
                                rhs=rhs,
                                start=(kc == 0 and k == 0),
                                stop=(kc == kc_chunks - 1 and k == ko_per_kc - 1),
                            )
                for t_sub in range(tsub_n):
                    ot = out_pool.tile([P, o_shard], f32, name="ot")
                    nc.vector.tensor_add(ot[:], psums[t_sub][:], bias_bc[:])
                    nc.sync.dma_start(
                        out=out[ds(m4 * tchunk + t_sub * P, P), :], in_=ot[:]
                    )

    nc.compile()
    return nc


def _io_np(mm_mode):
    if mm_mode == "bf16":
        import ml_dtypes

        return np.dtype(ml_dtypes.bfloat16)
    return np.dtype(np.float32)


def _to_fp32r(a):
    """RNE-round fp32 to the fp32r format (11-bit mantissa, low 12 bits 0)."""
    u = np.ascontiguousarray(a).view(np.uint32)
    lsb = (u >> np.uint32(12)) & np.uint32(1)
    r = (u + np.uint32(0x7FF) + lsb) & np.uint32(0xFFFFF000)
    return r.view(np.float32)


def make_in_maps(x, weight_mu, weight_rho, bias_mu, bias_rho, weight_epsilon,
                 bias_epsilon, mm_mode=MM_MODE, ncores=NCORES):
    io_np = _io_np(mm_mode)
    o_shard = weight_mu.shape[0] // ncores

    xT = np.ascontiguousarray(np.asarray(x, dtype=np.float32).T).astype(io_np)
    if mm_mode == "f32r":
        xT = _to_fp32r(xT)
    muT = np.ascontiguousarray(np.asarray(weight_mu, dtype=np.float32).T)
    rhoT = np.ascontiguousarray(np.asarray(weight_rho, dtype=np.float32).T)
    epsT = np.ascontiguousarray(np.asarray(weight_epsilon, dtype=np.float32).T)
    bmu = np.asarray(bias_mu, dtype=np.float32)
    brho = np.asarray(bias_rho, dtype=np.float32)
    beps = np.asarray(bias_epsilon, dtype=np.float32)

    in_maps = []
    for c in range(ncores):
        sl = slice(c * o_shard, (c + 1) * o_shard)
        wp = np.ascontiguousarray(
            np.stack([muT[:, sl], rhoT[:, sl], epsT[:, sl]], axis=1)
        ).astype(io_np)  # [IN, 3, O]
        b3 = np.stack([bmu[sl], brho[sl], beps[sl]], axis=0)  # [3, O]
        bp = np.ascontiguousarray(
            np.broadcast_to(b3[None], (P, 3, o_shard))
        ).astype(np.float32)
        in_maps.append({"xT": xT, "wp": wp, "bp": bp})
    return in_maps


def kernel(x, weight_mu, weight_rho, bias_mu, bias_rho, weight_epsilon,
           bias_epsilon):
    nc = build_nc(MM_MODE)
    in_maps = make_in_maps(
        x, weight_mu, weight_rho, bias_mu, bias_rho, weight_epsilon,
        bias_epsilon, MM_MODE,
    )
    res = run_bass_kernel_spmd(nc, in_maps, list(range(NCORES)))
    return np.concatenate(
        [res.results[i]["out"] for i in range(NCORES)], axis=1
    ).astype(np.float32)
